# revision 75
# baseline (speedup 1.0000x reference)
"""GraphSAGE 2-layer forward on 8 Trainium2 NeuronCores (v5: no collectives).

Strategy (per core, SPMD; all per-core variation is input data):
- Core c computes L1 for dst rows [c*125, (c+1)*125). It computes layer-0
  h ONLY for the rows its own L1 edges reference (unique(e1_src of its
  edges) + its 125 self rows, ~1250 rows -> 10 windows of 128). This
  duplicates ~48% of layer-0 work across cores but needs ZERO
  cross-core communication: no collectives, no pre-collective runtime
  barrier (~60 us), no exchange latency.
- L0 edge gather is done ON HOST: fp8 x rows pre-gathered in edge order
  (dst-sorted) into a partition-major stream; each 128-edge tile
  carries 602 B of features + a 128 B host-built one-hot (value 1/cnt)
  -> 730 B per tile per partition. The device streams it through a
  rotating SBUF buffer with linear HWDGE DMAs, consumer-paced.
- Aggregation: PE accumulates aggT[featchunk,dst] += G.T @ OH in PSUM
  per 128-row window; h = relu(xselfT @ [Wself;b] + aggT @ Wneigh) with
  xselfT a host-packed transposed x block of the core's rows. Dense
  matmuls for window w are deferred until after window w+1's agg tiles
  (double-buffered ps_agg/ps_h/aggT) so the PE never stalls on the
  scalar PSUM->SBUF copies.
- h stays SBUF-resident. L1: per-window one-hot matmuls against h_sb
  (lhsT = h window, rhs = host-built fp16 one-hot with 1/cnt values,
  multi-edge rows folded); self tile via identity one-hot on window 0
  (self rows pinned to slots 0..124); out[125, 41] fp32 per core,
  concatenated on host.
"""

import numpy as np

P = 128
NCORES = 8

N_SRC0, N_DST0, N_E0 = 286000, 11000, 275000
N_DST1, N_E1 = 1000, 10000
F_IN, N_HID, N_CLS = 602, 256, 41
TROW = F_IN + P          # 730 B per tile per partition: 602 G + 128 OH
GO_R = 10                # go-stream chunk slots / sem rotation


def _chunks(k):
    out = []
    while k > 0:
        out.append(min(P, k))
        k -= P
    return out


def _preprocess(x, Wself0, Wneigh0, b0, Wself1, Wneigh1, b1,
                e0_src, e0_dst, e1_src, e1_dst):
    e0_src = np.asarray(e0_src).astype(np.int64)
    e0_dst = np.asarray(e0_dst).astype(np.int64)
    e1_src = np.asarray(e1_src).astype(np.int64)
    e1_dst = np.asarray(e1_dst).astype(np.int64)
    x = np.asarray(x, dtype=np.float32)

    dpc1 = N_DST1 // NCORES
    cnt0 = np.bincount(e0_dst, minlength=N_DST0).astype(np.float64)
    cntinv0 = (1.0 / np.maximum(cnt0, 1.0)).astype(np.float32)
    cnt1 = np.bincount(e1_dst, minlength=N_DST1).astype(np.float64)
    cntinv1 = (1.0 / np.maximum(cnt1, 1.0)).astype(np.float32)

    core1 = e1_dst // dpc1

    # per-core row sets (self rows + L1-referenced rows)
    rowlists, rowpos = [], []
    nwc = 0
    for c in range(NCORES):
        selfs = np.arange(c * dpc1, (c + 1) * dpc1)
        uniq = np.unique(e1_src[core1 == c])
        others = np.setdiff1d(uniq, selfs)
        nwc = max(nwc, -(-(dpc1 + len(others)) // P))
        rowlists.append((selfs, others))
    NWC = nwc

    # window assignment per core: self rows pinned to window 0 slots
    # 0..124; remaining rows dealt greedily by L0 degree into windows
    rl_full = []
    for c in range(NCORES):
        selfs, others = rowlists[c]
        slots = [[] for _ in range(NWC)]
        cap = [P] * NWC
        slots[0] = list(selfs)
        wload = np.zeros(NWC, np.float64)
        wload[0] = cnt0[selfs].sum()
        for u in sorted(others, key=lambda u: -cnt0[u]):
            cands = [w for w in range(NWC) if len(slots[w]) < cap[w]]
            w = min(cands, key=lambda ww: wload[ww])
            slots[w].append(u)
            wload[w] += cnt0[u]
        rl = np.full(NWC * P, -1, np.int64)
        for w in range(NWC):
            rl[w * P: w * P + len(slots[w])] = slots[w]
        rl_full.append(rl)
        pos = np.full(N_DST0, -1, np.int64)
        val = rl >= 0
        pos[rl[val]] = np.where(val)[0]
        rowpos.append(pos)

    # per-(core, window) L0 edge lists
    percw = {}
    for c in range(NCORES):
        sl = rowpos[c][e0_dst]
        keep = sl >= 0
        s0, p0, d0 = e0_src[keep], sl[keep], e0_dst[keep]
        o = np.argsort(p0, kind="stable")
        s0, p0, d0 = s0[o], p0[o], d0[o]
        w0 = p0 // P
        for wi in range(NWC):
            m = w0 == wi
            percw[(c, wi)] = (s0[m], p0[m] - wi * P, d0[m])

    tiles_w0 = [max(1, max(-(-len(percw[(c, wi)][0]) // P)
                           for c in range(NCORES))) for wi in range(NWC)]
    ntiles0 = sum(tiles_w0)
    cum_w0 = np.cumsum([0] + tiles_w0)

    x16 = x.astype(np.float16)
    ch0 = _chunks(F_IN)
    NC0 = len(ch0)
    SFW = NWC * P

    # go-stream DMA chunking: fine chunks early, then half-windows
    def _split(a, b, n):
        cuts = [a + (b - a) * i // n for i in range(n + 1)]
        return [(cuts[i], cuts[i + 1]) for i in range(n) if cuts[i + 1] > cuts[i]]
    go_dmas = []
    go_dmas += _split(0, 3, 1)
    go_dmas += _split(3, int(cum_w0[1]), 3)
    go_dmas += _split(int(cum_w0[1]), int(cum_w0[2]), 3)
    for w in range(2, NWC):
        go_dmas += _split(int(cum_w0[w]), int(cum_w0[w + 1]), 2)
    gate_of_tile = {}
    slot_of_chunk = {}
    for gi, (a, b) in enumerate(go_dmas):
        slot_of_chunk[gi] = gi % GO_R
        for t in range(a, b):
            gate_of_tile[t] = gi + 1
    # rotating GO buffer slot layout: slot size = max chunk tiles
    slot_tiles = max(b - a for a, b in go_dmas)

    in_maps = []
    for c in range(NCORES):
        # --- L0 host-gathered edge stream (chunk-slot padded) ---
        go = np.zeros((P, ntiles0, TROW), dtype=np.float16)
        for wi in range(NWC):
            es, eslot, edor = percw[(c, wi)]
            ne = len(es)
            t0 = int(cum_w0[wi])
            tloc = np.arange(ne) // P + t0
            ploc = np.arange(ne) % P
            go[ploc, tloc, :F_IN] = x16[es]
            go[ploc, tloc, F_IN + eslot] = cntinv0[edor]
        go8 = go.reshape(P, ntiles0 * TROW).astype("float8_e4m3")

        # --- transposed x block for the dense self path ---
        rl = rl_full[c]
        xst = np.zeros((P, NC0 * SFW), np.float16)
        val = rl >= 0
        xs = np.zeros((NWC * P, F_IN), np.float16)
        xs[val] = x16[rl[val]]
        for cc in range(NC0):
            kc = ch0[cc]
            xst[:kc, cc * SFW:(cc + 1) * SFW] = xs[:, cc * P: cc * P + kc].T
        xst[ch0[-1], (NC0 - 1) * SFW: NC0 * SFW] = 1.0

        # --- L1 one-hots: [self | window 0..NWC-1] tiles ---
        oh1f = np.zeros((P, (NWC + 1) * P), np.float32)
        oh1f[np.arange(dpc1), np.arange(dpc1)] = 1.0     # self tile
        m = core1 == c
        s1, d1 = e1_src[m], e1_dst[m]
        pos = rowpos[c][s1]
        assert (pos >= 0).all()
        wv, sv = pos // P, pos % P
        np.add.at(oh1f, (sv, (1 + wv) * P + (d1 - c * dpc1)), cntinv1[d1])
        oh1 = oh1f.astype(np.float16)

        in_maps.append({
            "go": go8, "xselfT": xst, "oh1": oh1,
            "ones1_in": np.ones((1, P), np.float16),
        })

    W0s = np.concatenate([np.asarray(Wself0, np.float32),
                          np.asarray(b0, np.float32)[None, :]], 0).astype(np.float16)
    W0n = np.asarray(Wneigh0, np.float32).astype(np.float16)
    W1s = np.concatenate([np.asarray(Wself1, np.float32),
                          np.asarray(b1, np.float32)[None, :]], 0).astype(np.float16)
    W1n = np.asarray(Wneigh1, np.float32).astype(np.float16)
    for m2 in in_maps:
        m2.update({"W0s": W0s, "W0n": W0n, "W1s": W1s, "W1n": W1n})

    params = dict(
        nwc=NWC, dpc1=dpc1, tiles_w0=tiles_w0, ntiles0=ntiles0,
        go_dmas=go_dmas, gate_of_tile=gate_of_tile, slot_tiles=slot_tiles,
    )
    return in_maps, params


def _build_nc(prm):
    import concourse.bacc as bacc
    import concourse.mybir as mybir

    f_in, n_hid, n_cls = F_IN, N_HID, N_CLS
    dpc1 = prm["dpc1"]
    nwc = prm["nwc"]
    tiles_w0 = prm["tiles_w0"]
    ntiles0 = prm["ntiles0"]
    go_dmas = prm["go_dmas"]
    gate_of_tile = prm["gate_of_tile"]
    slot_tiles = prm["slot_tiles"]

    ch0 = _chunks(f_in)
    ch1 = _chunks(n_hid)
    NC0, NC1 = len(ch0), len(ch1)
    FPAD0 = NC0 * P
    SFW = nwc * P
    cum_w0 = np.cumsum([0] + tiles_w0)
    cum_tiles = [int(v) for v in cum_w0]
    # tile -> (chunk, offset-within-chunk) for the rotating GO buffer
    chunk_of_tile = {}
    for gi, (a, b) in enumerate(go_dmas):
        for t in range(a, b):
            chunk_of_tile[t] = (gi, t - a)

    banks0 = [(c * P * 4) // 2048 for c in range(NC0)]
    first_c0 = {b: min(c for c in range(NC0) if banks0[c] == b) for b in set(banks0)}
    last_c0 = {b: max(c for c in range(NC0) if banks0[c] == b) for b in set(banks0)}

    nc = bacc.Bacc("TRN2", target_bir_lowering=False, debug=False,
                   num_devices=NCORES, dynamic_dma_scratch_size=2**14)
    dt = mybir.dt
    AF = mybir.ActivationFunctionType

    go_d = nc.dram_tensor("go", [P, ntiles0 * TROW], dt.float8e4, kind="ExternalInput")
    xselfT_d = nc.dram_tensor("xselfT", [P, NC0 * SFW], dt.float16, kind="ExternalInput")
    oh1_d = nc.dram_tensor("oh1", [P, (nwc + 1) * P], dt.float16, kind="ExternalInput")
    W0s_d = nc.dram_tensor("W0s", [f_in + 1, n_hid], dt.float16, kind="ExternalInput")
    W0n_d = nc.dram_tensor("W0n", [f_in, n_hid], dt.float16, kind="ExternalInput")
    W1s_d = nc.dram_tensor("W1s", [n_hid + 1, n_cls], dt.float16, kind="ExternalInput")
    W1n_d = nc.dram_tensor("W1n", [n_hid, n_cls], dt.float16, kind="ExternalInput")
    ones1_d = nc.dram_tensor("ones1_in", [1, P], dt.float16, kind="ExternalInput")
    out_d = nc.dram_tensor("out", [P, n_cls], dt.float32, kind="ExternalOutput")

    from contextlib import ExitStack
    es = ExitStack()
    with es:
        block = es.enter_context(nc.Block())
        sem = lambda n: es.enter_context(nc.semaphore(n))
        sb = lambda n, shp, d: es.enter_context(nc.sbuf_tensor(n, shp, d))
        ps = lambda n, shp: es.enter_context(nc.psum_tensor(n, shp, dt.float32))
        (s_init, s_ini1b, s_ini2, s_pe, s_l1, s_cp, s_wmm, s_hs, s_od) = (
            sem("s_init"), sem("s_ini1b"), sem("s_ini2"), sem("s_pe"),
            sem("s_l1"), sem("s_cp"), sem("s_wmm"), sem("s_hs"), sem("s_od"))
        s_goN = [sem(f"s_go{i}") for i in range(GO_R)]
        GO = sb("GO", [P, GO_R * slot_tiles * TROW], dt.float8e4)
        OH1 = sb("OH1", [P, (nwc + 1) * P], dt.float16)
        xselfT = sb("xselfT_s", [P, NC0 * SFW], dt.float16)
        W0s_s = sb("W0s_s", [P, NC0 * n_hid], dt.float16)
        W0n_s = sb("W0n_s", [P, NC0 * n_hid], dt.float16)
        W1s_s = sb("W1s_s", [P, NC1 * n_cls], dt.float16)
        W1n_s = sb("W1n_s", [P, NC1 * n_cls], dt.float16)
        b1row = sb("b1row", [1, n_cls], dt.float16)
        ones1 = sb("ones1", [1, P], dt.float16)
        aggT = sb("aggT", [P, 2 * FPAD0], dt.float16)
        agg1T = sb("agg1T", [P, NC1 * P], dt.float16)
        self1T = sb("self1T", [P, NC1 * P], dt.float16)
        h_sb = sb("h_sb", [P, nwc * n_hid], dt.float16)
        out_sb = sb("out_sb", [P, n_cls], dt.float32)
        ps_agg = [ps("ps_aggA", [P, FPAD0]), ps("ps_aggB", [P, FPAD0])]
        ps_h = [ps("ps_hA", [P, n_hid]), ps("ps_hB", [P, n_hid])]
        ps_l1 = ps("ps_l1", [P, 2 * NC1 * P])    # [agg1 0:256 | self1 256:512]
        ps_out = ps("ps_out", [P, n_cls])

        n_init = 0
        n_ini1b = 0
        n_ini2 = 0
        WSPLIT = (nwc + 1) // 2   # xselfT windows < WSPLIT load early

        @block.sync
        def _(sp):
            nonlocal n_init, n_ini1b, n_ini2
            # edge/onehot stream through the rotating GO buffer; dense-path
            # weights + first xselfT half after chunk 3, the rest after
            # chunk 7. Consumer-paced issue keeps rotating-sem waits sound.
            for gi, (a, b) in enumerate(go_dmas):
                if gi >= GO_R:
                    sp.wait_ge(s_pe, go_dmas[gi - GO_R][1])
                sl = (gi % GO_R) * slot_tiles
                sp.dma_start(out=GO[:, sl * TROW: (sl + b - a) * TROW],
                             in_=go_d[:, a * TROW: b * TROW]
                             ).then_inc(s_goN[gi % GO_R], 16)
                if gi == 3:
                    def ld(dst_ap, src_ap):
                        nonlocal n_init
                        sp.dma_start(out=dst_ap, in_=src_ap).then_inc(s_init, 16)
                        n_init += 1
                    ofs = 0
                    for c, kc in enumerate(ch0):
                        ld(W0s_s[0:kc, c * n_hid:(c + 1) * n_hid], W0s_d[ofs:ofs + kc, :])
                        ld(W0n_s[0:kc, c * n_hid:(c + 1) * n_hid], W0n_d[ofs:ofs + kc, :])
                        ofs += kc
                    last = NC0 - 1
                    ld(W0s_s[ch0[last]:ch0[last] + 1, last * n_hid:(last + 1) * n_hid],
                       W0s_d[f_in:f_in + 1, :])
                    for c in range(NC0):
                        kcr = ch0[c] + (1 if c == NC0 - 1 else 0)
                        ld(xselfT[0:kcr, c * SFW: c * SFW + P],
                           xselfT_d[0:kcr, c * SFW: c * SFW + P])
                    ld(OH1[:, :], oh1_d[:, :])
                elif gi == 5:
                    def ld1b(dst_ap, src_ap):
                        nonlocal n_ini1b
                        sp.dma_start(out=dst_ap, in_=src_ap).then_inc(s_ini1b, 16)
                        n_ini1b += 1
                    for c in range(NC0):
                        kcr = ch0[c] + (1 if c == NC0 - 1 else 0)
                        ld1b(xselfT[0:kcr, c * SFW + P: c * SFW + WSPLIT * P],
                             xselfT_d[0:kcr, c * SFW + P: c * SFW + WSPLIT * P])
                elif gi == 7:
                    def ld2(dst_ap, src_ap):
                        nonlocal n_ini2
                        sp.dma_start(out=dst_ap, in_=src_ap).then_inc(s_ini2, 16)
                        n_ini2 += 1
                    for c in range(NC0):
                        kcr = ch0[c] + (1 if c == NC0 - 1 else 0)
                        ld2(xselfT[0:kcr, c * SFW + WSPLIT * P: (c + 1) * SFW],
                            xselfT_d[0:kcr, c * SFW + WSPLIT * P: (c + 1) * SFW])
                    ofs = 0
                    for c, kc in enumerate(ch1):
                        ld2(W1s_s[0:kc, c * n_cls:(c + 1) * n_cls], W1s_d[ofs:ofs + kc, :])
                        ld2(W1n_s[0:kc, c * n_cls:(c + 1) * n_cls], W1n_d[ofs:ofs + kc, :])
                        ofs += kc
                    ld2(b1row[0:1, :], W1s_d[n_hid:n_hid + 1, :])
                    ld2(ones1[0:1, :], ones1_d[0:1, :])
            sp.wait_ge(s_od, 16)

        def dense0(t_, w):
            """dense matmuls producing h window w (into ps_h[w%2])"""
            t_.wait_ge(s_cp, NC0 * (w + 1))      # copies of window w done
            if w >= 2:
                t_.wait_ge(s_hs, w - 1)          # ps_h[w%2] free (relu w-2 done)
            bb = w % 2
            k = 0
            for c in range(NC0):
                kc = ch0[c] + (1 if c == NC0 - 1 else 0)
                t_.matmul(out=ps_h[bb][0:P, 0:n_hid],
                          lhsT=xselfT[0:kc, c * SFW + w * P: c * SFW + (w + 1) * P],
                          rhs=W0s_s[0:kc, c * n_hid:(c + 1) * n_hid],
                          start=(k == 0), stop=False)
                k += 1
            for c in range(NC0):
                kc = ch0[c]
                mm = t_.matmul(out=ps_h[bb][0:P, 0:n_hid],
                               lhsT=aggT[0:kc, bb * FPAD0 + c * P: bb * FPAD0 + (c + 1) * P],
                               rhs=W0n_s[0:kc, c * n_hid:(c + 1) * n_hid],
                               start=False, stop=(k == 2 * NC0 - 1))
                k += 1
            mm.then_inc(s_wmm, 1)

        def l1tile(t_, j):
            """L1 one-hot matmul tile j: j=0 self (identity), else window j-1"""
            t_.wait_ge(s_hs, max(1, j))          # its window's relu done
            base = NC1 * P if j == 0 else 0
            hofs = 0 if j == 0 else (j - 1) * n_hid
            for c in range(NC1):
                mm = t_.matmul(
                    out=ps_l1[0:P, base + c * P: base + (c + 1) * P],
                    lhsT=h_sb[0:P, hofs + c * P: hofs + (c + 1) * P],
                    rhs=OH1[:, j * P:(j + 1) * P],
                    start=(j == 0 and c == 0),
                    stop=(j == nwc and c == NC1 - 1))
            mm.then_inc(s_l1, 1)

        @block.tensor
        def _(t_):
            gate = 0
            for w in range(nwc):
                bb = w % 2
                if w >= 2:
                    t_.wait_ge(s_cp, NC0 * (w - 1))   # ps_agg[bb] free
                for j in range(tiles_w0[w]):
                    t = cum_tiles[w] + j
                    if gate_of_tile[t] > gate:
                        gate = gate_of_tile[t]
                        gc_ = gate - 1
                        t_.wait_ge(s_goN[gc_ % GO_R], 16 * (gc_ // GO_R + 1))
                    gi, toff = chunk_of_tile[t]
                    base = ((gi % GO_R) * slot_tiles + toff) * TROW
                    first = (j == 0)
                    lastt = (j == tiles_w0[w] - 1)
                    fofs = 0
                    for c in range(NC0):
                        mc = ch0[c]
                        mm = t_.matmul(
                            out=ps_agg[bb][0:mc, c * P:(c + 1) * P],
                            lhsT=GO[:, base + fofs: base + fofs + mc],
                            rhs=GO[:, base + F_IN: base + TROW],
                            start=first and (c == first_c0[banks0[c]]),
                            stop=lastt and (c == last_c0[banks0[c]]))
                        fofs += mc
                    mm.then_inc(s_pe, 1)
                if w == 0:
                    t_.wait_ge(s_init, 16 * n_init)
                if w == 2:
                    t_.wait_ge(s_ini1b, 16 * n_ini1b)  # xselfT w1..5 in
                if w == WSPLIT + 1:
                    t_.wait_ge(s_ini2, 16 * n_ini2)   # 2nd xselfT half in
                if w >= 1:
                    dense0(t_, w - 1)
                if w >= 2:
                    l1tile(t_, w - 2)            # interleaved: relu ready
            dense0(t_, nwc - 1)
            for j in range(nwc - 2, nwc + 1):    # remaining L1 tiles
                l1tile(t_, j)
            # L1 dense
            t_.wait_ge(s_cp, NC0 * nwc + 2 * NC1)
            k = 0
            nmm = 2 * NC1 + 1
            for c in range(NC1):
                mc = ch1[c]
                t_.matmul(out=ps_out[0:dpc1, 0:n_cls],
                          lhsT=self1T[0:mc, c * P: c * P + dpc1],
                          rhs=W1s_s[0:mc, c * n_cls:(c + 1) * n_cls],
                          start=(k == 0), stop=False)
                k += 1
            t_.matmul(out=ps_out[0:dpc1, 0:n_cls],
                      lhsT=ones1[0:1, 0:dpc1],
                      rhs=b1row[0:1, 0:n_cls],
                      start=False, stop=False)
            k += 1
            for c in range(NC1):
                mc = ch1[c]
                mm = t_.matmul(out=ps_out[0:dpc1, 0:n_cls],
                               lhsT=agg1T[0:mc, c * P: c * P + dpc1],
                               rhs=W1n_s[0:mc, c * n_cls:(c + 1) * n_cls],
                               start=False, stop=(k == nmm - 1))
                k += 1
            mm.then_inc(s_wmm, 1)

        @block.scalar
        def _(s):
            for w in range(nwc):
                bb = w % 2
                s.wait_ge(s_pe, cum_tiles[w + 1])
                for c in range(NC0):
                    mc = ch0[c]
                    s.activation(out=aggT[0:mc, bb * FPAD0 + c * P: bb * FPAD0 + (c + 1) * P],
                                 in_=ps_agg[bb][0:mc, c * P:(c + 1) * P],
                                 func=AF.Copy).then_inc(s_cp, 1)
                if w >= 1:
                    s.wait_ge(s_wmm, w)
                    s.activation(out=h_sb[:, (w - 1) * n_hid: w * n_hid],
                                 in_=ps_h[(w - 1) % 2][:, :], func=AF.Relu).then_inc(s_hs, 1)
            w = nwc
            s.wait_ge(s_wmm, w)
            s.activation(out=h_sb[:, (w - 1) * n_hid: w * n_hid],
                         in_=ps_h[(w - 1) % 2][:, :], func=AF.Relu).then_inc(s_hs, 1)
            # L1 copies
            s.wait_ge(s_l1, nwc + 1)
            for c in range(NC1):
                s.activation(out=agg1T[0:P, c * P:(c + 1) * P],
                             in_=ps_l1[0:P, c * P:(c + 1) * P],
                             func=AF.Copy).then_inc(s_cp, 1)
                s.activation(out=self1T[0:P, c * P:(c + 1) * P],
                             in_=ps_l1[0:P, NC1 * P + c * P: NC1 * P + (c + 1) * P],
                             func=AF.Copy).then_inc(s_cp, 1)
            s.wait_ge(s_wmm, nwc + 1)
            s.activation(out=out_sb[0:dpc1, :], in_=ps_out[0:dpc1, :],
                         func=AF.Copy).then_inc(s_hs, 1)
            s.wait_ge(s_hs, nwc + 1)   # out_sb writes landed
            s.dma_start(out=out_d[0:dpc1, :], in_=out_sb[0:dpc1, :]).then_inc(s_od, 16)

    nc.compile()
    return nc, None


def _run(inputs, dims=None, trace=False, tmpdir=None):
    from concourse.bass_utils import run_bass_kernel_spmd
    in_maps, prm = _preprocess(**inputs)
    nc, _ = _build_nc(prm)
    res = run_bass_kernel_spmd(nc, in_maps, core_ids=list(range(NCORES)),
                               trace=trace, tmpdir=tmpdir)
    dpc1 = N_DST1 // NCORES
    out = np.concatenate([res.results[c]["out"][:dpc1] for c in range(NCORES)], 0)
    return out.astype(np.float32), res


def kernel(**inputs):
    out, _ = _run(inputs)
    return out


# revision 77
# speedup vs baseline: 1.0137x; 1.0137x over previous
"""GraphSAGE 2-layer forward on 8 Trainium2 NeuronCores (v5: no collectives).

Strategy (per core, SPMD; all per-core variation is input data):
- Core c computes L1 for dst rows [c*125, (c+1)*125). It computes layer-0
  h ONLY for the rows its own L1 edges reference (unique(e1_src of its
  edges) + its 125 self rows, ~1250 rows -> 10 windows of 128). This
  duplicates ~48% of layer-0 work across cores but needs ZERO
  cross-core communication: no collectives, no pre-collective runtime
  barrier (~60 us), no exchange latency.
- L0 edge gather is done ON HOST: fp8 x rows pre-gathered in edge order
  (dst-sorted) into a partition-major stream; each 128-edge tile
  carries 602 B of features + a 128 B host-built one-hot (value 1/cnt)
  -> 730 B per tile per partition. The device streams it through a
  rotating SBUF buffer with linear HWDGE DMAs, consumer-paced.
- Aggregation: PE accumulates aggT[featchunk,dst] += G.T @ OH in PSUM
  per 128-row window; h = relu(xselfT @ [Wself;b] + aggT @ Wneigh) with
  xselfT a host-packed transposed x block of the core's rows. Dense
  matmuls for window w are deferred until after window w+1's agg tiles
  (double-buffered ps_agg/ps_h/aggT) so the PE never stalls on the
  scalar PSUM->SBUF copies.
- h stays SBUF-resident. L1: per-window one-hot matmuls against h_sb
  (lhsT = h window, rhs = host-built fp16 one-hot with 1/cnt values,
  multi-edge rows folded); self tile via identity one-hot on window 0
  (self rows pinned to slots 0..124); out[125, 41] fp32 per core,
  concatenated on host.
"""

import numpy as np

P = 128
NCORES = 8

N_SRC0, N_DST0, N_E0 = 286000, 11000, 275000
N_DST1, N_E1 = 1000, 10000
F_IN, N_HID, N_CLS = 602, 256, 41
TROW = F_IN + P          # 730 B per tile per partition: 602 G + 128 OH
GO_R = 6                 # go-stream chunk slots / sem rotation


def _chunks(k):
    out = []
    while k > 0:
        out.append(min(P, k))
        k -= P
    return out


def _preprocess(x, Wself0, Wneigh0, b0, Wself1, Wneigh1, b1,
                e0_src, e0_dst, e1_src, e1_dst):
    e0_src = np.asarray(e0_src).astype(np.int64)
    e0_dst = np.asarray(e0_dst).astype(np.int64)
    e1_src = np.asarray(e1_src).astype(np.int64)
    e1_dst = np.asarray(e1_dst).astype(np.int64)
    x = np.asarray(x, dtype=np.float32)

    dpc1 = N_DST1 // NCORES
    cnt0 = np.bincount(e0_dst, minlength=N_DST0).astype(np.float64)
    cntinv0 = (1.0 / np.maximum(cnt0, 1.0)).astype(np.float32)
    cnt1 = np.bincount(e1_dst, minlength=N_DST1).astype(np.float64)
    cntinv1 = (1.0 / np.maximum(cnt1, 1.0)).astype(np.float32)

    core1 = e1_dst // dpc1

    # per-core row sets (self rows + L1-referenced rows)
    rowlists, rowpos = [], []
    nwc = 0
    for c in range(NCORES):
        selfs = np.arange(c * dpc1, (c + 1) * dpc1)
        uniq = np.unique(e1_src[core1 == c])
        others = np.setdiff1d(uniq, selfs)
        nwc = max(nwc, -(-(dpc1 + len(others)) // P))
        rowlists.append((selfs, others))
    NWC = nwc

    # window assignment per core: self rows pinned to window 0 slots
    # 0..124; remaining rows dealt greedily by L0 degree into windows
    rl_full = []
    for c in range(NCORES):
        selfs, others = rowlists[c]
        slots = [[] for _ in range(NWC)]
        cap = [P] * NWC
        slots[0] = list(selfs)
        wload = np.zeros(NWC, np.float64)
        wload[0] = cnt0[selfs].sum()
        for u in sorted(others, key=lambda u: -cnt0[u]):
            cands = [w for w in range(NWC) if len(slots[w]) < cap[w]]
            w = min(cands, key=lambda ww: wload[ww])
            slots[w].append(u)
            wload[w] += cnt0[u]
        rl = np.full(NWC * P, -1, np.int64)
        for w in range(NWC):
            rl[w * P: w * P + len(slots[w])] = slots[w]
        rl_full.append(rl)
        pos = np.full(N_DST0, -1, np.int64)
        val = rl >= 0
        pos[rl[val]] = np.where(val)[0]
        rowpos.append(pos)

    # per-(core, window) L0 edge lists
    percw = {}
    for c in range(NCORES):
        sl = rowpos[c][e0_dst]
        keep = sl >= 0
        s0, p0, d0 = e0_src[keep], sl[keep], e0_dst[keep]
        o = np.argsort(p0, kind="stable")
        s0, p0, d0 = s0[o], p0[o], d0[o]
        w0 = p0 // P
        for wi in range(NWC):
            m = w0 == wi
            percw[(c, wi)] = (s0[m], p0[m] - wi * P, d0[m])

    tiles_w0 = [max(1, max(-(-len(percw[(c, wi)][0]) // P)
                           for c in range(NCORES))) for wi in range(NWC)]
    ntiles0 = sum(tiles_w0)
    cum_w0 = np.cumsum([0] + tiles_w0)

    x16 = x.astype(np.float16)
    ch0 = _chunks(F_IN)
    NC0 = len(ch0)
    SFW = NWC * P

    # go-stream DMA chunking: fine chunks early, then half-windows
    def _split(a, b, n):
        cuts = [a + (b - a) * i // n for i in range(n + 1)]
        return [(cuts[i], cuts[i + 1]) for i in range(n) if cuts[i + 1] > cuts[i]]
    go_dmas = []
    for w in range(NWC):
        go_dmas += _split(int(cum_w0[w]), int(cum_w0[w + 1]), 1)
    gate_of_tile = {}
    slot_of_chunk = {}
    for gi, (a, b) in enumerate(go_dmas):
        slot_of_chunk[gi] = gi % GO_R
        for t in range(a, b):
            gate_of_tile[t] = gi + 1
    # rotating GO buffer slot layout: slot size = max chunk tiles
    slot_tiles = max(b - a for a, b in go_dmas)

    in_maps = []
    for c in range(NCORES):
        # --- L0 host-gathered edge stream (chunk-slot padded) ---
        go = np.zeros((P, ntiles0, TROW), dtype=np.float16)
        for wi in range(NWC):
            es, eslot, edor = percw[(c, wi)]
            ne = len(es)
            t0 = int(cum_w0[wi])
            tloc = np.arange(ne) // P + t0
            ploc = np.arange(ne) % P
            go[ploc, tloc, :F_IN] = x16[es]
            go[ploc, tloc, F_IN + eslot] = cntinv0[edor]
        go8 = go.reshape(P, ntiles0 * TROW).astype("float8_e4m3")

        # --- transposed x block for the dense self path ---
        rl = rl_full[c]
        xst = np.zeros((P, NC0 * SFW), np.float16)
        val = rl >= 0
        xs = np.zeros((NWC * P, F_IN), np.float16)
        xs[val] = x16[rl[val]]
        for cc in range(NC0):
            kc = ch0[cc]
            xst[:kc, cc * SFW:(cc + 1) * SFW] = xs[:, cc * P: cc * P + kc].T
        xst[ch0[-1], (NC0 - 1) * SFW: NC0 * SFW] = 1.0

        # --- L1 one-hots: [self | window 0..NWC-1] tiles ---
        oh1f = np.zeros((P, (NWC + 1) * P), np.float32)
        oh1f[np.arange(dpc1), np.arange(dpc1)] = 1.0     # self tile
        m = core1 == c
        s1, d1 = e1_src[m], e1_dst[m]
        pos = rowpos[c][s1]
        assert (pos >= 0).all()
        wv, sv = pos // P, pos % P
        np.add.at(oh1f, (sv, (1 + wv) * P + (d1 - c * dpc1)), cntinv1[d1])
        oh1 = oh1f.astype(np.float16)

        in_maps.append({
            "go": go8, "xselfT": xst, "oh1": oh1,
            "ones1_in": np.ones((1, P), np.float16),
        })

    W0s = np.concatenate([np.asarray(Wself0, np.float32),
                          np.asarray(b0, np.float32)[None, :]], 0).astype(np.float16)
    W0n = np.asarray(Wneigh0, np.float32).astype(np.float16)
    W1s = np.concatenate([np.asarray(Wself1, np.float32),
                          np.asarray(b1, np.float32)[None, :]], 0).astype(np.float16)
    W1n = np.asarray(Wneigh1, np.float32).astype(np.float16)
    for m2 in in_maps:
        m2.update({"W0s": W0s, "W0n": W0n, "W1s": W1s, "W1n": W1n})

    params = dict(
        nwc=NWC, dpc1=dpc1, tiles_w0=tiles_w0, ntiles0=ntiles0,
        go_dmas=go_dmas, gate_of_tile=gate_of_tile, slot_tiles=slot_tiles,
    )
    return in_maps, params


def _build_nc(prm):
    import concourse.bacc as bacc
    import concourse.mybir as mybir

    f_in, n_hid, n_cls = F_IN, N_HID, N_CLS
    dpc1 = prm["dpc1"]
    nwc = prm["nwc"]
    tiles_w0 = prm["tiles_w0"]
    ntiles0 = prm["ntiles0"]
    go_dmas = prm["go_dmas"]
    gate_of_tile = prm["gate_of_tile"]
    slot_tiles = prm["slot_tiles"]

    ch0 = _chunks(f_in)
    ch1 = _chunks(n_hid)
    NC0, NC1 = len(ch0), len(ch1)
    FPAD0 = NC0 * P
    SFW = nwc * P
    cum_w0 = np.cumsum([0] + tiles_w0)
    cum_tiles = [int(v) for v in cum_w0]
    # tile -> (chunk, offset-within-chunk) for the rotating GO buffer
    chunk_of_tile = {}
    for gi, (a, b) in enumerate(go_dmas):
        for t in range(a, b):
            chunk_of_tile[t] = (gi, t - a)

    banks0 = [(c * P * 4) // 2048 for c in range(NC0)]
    first_c0 = {b: min(c for c in range(NC0) if banks0[c] == b) for b in set(banks0)}
    last_c0 = {b: max(c for c in range(NC0) if banks0[c] == b) for b in set(banks0)}

    nc = bacc.Bacc("TRN2", target_bir_lowering=False, debug=False,
                   num_devices=NCORES, dynamic_dma_scratch_size=2**14)
    dt = mybir.dt
    AF = mybir.ActivationFunctionType

    go_d = nc.dram_tensor("go", [P, ntiles0 * TROW], dt.float8e4, kind="ExternalInput")
    xselfT_d = nc.dram_tensor("xselfT", [P, NC0 * SFW], dt.float16, kind="ExternalInput")
    oh1_d = nc.dram_tensor("oh1", [P, (nwc + 1) * P], dt.float16, kind="ExternalInput")
    W0s_d = nc.dram_tensor("W0s", [f_in + 1, n_hid], dt.float16, kind="ExternalInput")
    W0n_d = nc.dram_tensor("W0n", [f_in, n_hid], dt.float16, kind="ExternalInput")
    W1s_d = nc.dram_tensor("W1s", [n_hid + 1, n_cls], dt.float16, kind="ExternalInput")
    W1n_d = nc.dram_tensor("W1n", [n_hid, n_cls], dt.float16, kind="ExternalInput")
    ones1_d = nc.dram_tensor("ones1_in", [1, P], dt.float16, kind="ExternalInput")
    out_d = nc.dram_tensor("out", [P, n_cls], dt.float32, kind="ExternalOutput")

    from contextlib import ExitStack
    es = ExitStack()
    with es:
        block = es.enter_context(nc.Block())
        sem = lambda n: es.enter_context(nc.semaphore(n))
        sb = lambda n, shp, d: es.enter_context(nc.sbuf_tensor(n, shp, d))
        ps = lambda n, shp: es.enter_context(nc.psum_tensor(n, shp, dt.float32))
        (s_init, s_ini1b, s_ini2, s_pe, s_l1, s_cp, s_wmm, s_hs, s_od) = (
            sem("s_init"), sem("s_ini1b"), sem("s_ini2"), sem("s_pe"),
            sem("s_l1"), sem("s_cp"), sem("s_wmm"), sem("s_hs"), sem("s_od"))
        s_goN = [sem(f"s_go{i}") for i in range(GO_R)]
        GO = sb("GO", [P, GO_R * slot_tiles * TROW], dt.float8e4)
        OH1 = sb("OH1", [P, (nwc + 1) * P], dt.float16)
        xselfT = sb("xselfT_s", [P, NC0 * SFW], dt.float16)
        W0s_s = sb("W0s_s", [P, NC0 * n_hid], dt.float16)
        W0n_s = sb("W0n_s", [P, NC0 * n_hid], dt.float16)
        W1s_s = sb("W1s_s", [P, NC1 * n_cls], dt.float16)
        W1n_s = sb("W1n_s", [P, NC1 * n_cls], dt.float16)
        b1row = sb("b1row", [1, n_cls], dt.float16)
        ones1 = sb("ones1", [1, P], dt.float16)
        aggT = sb("aggT", [P, 2 * FPAD0], dt.float16)
        agg1T = sb("agg1T", [P, NC1 * P], dt.float16)
        self1T = sb("self1T", [P, NC1 * P], dt.float16)
        h_sb = sb("h_sb", [P, nwc * n_hid], dt.float16)
        out_sb = sb("out_sb", [P, n_cls], dt.float32)
        ps_agg = [ps("ps_aggA", [P, FPAD0]), ps("ps_aggB", [P, FPAD0])]
        ps_h = [ps("ps_hA", [P, n_hid]), ps("ps_hB", [P, n_hid])]
        ps_l1 = ps("ps_l1", [P, 2 * NC1 * P])    # [agg1 0:256 | self1 256:512]
        ps_out = ps("ps_out", [P, n_cls])

        n_init = 0
        n_ini1b = 0
        n_ini2 = 0
        WSPLIT = (nwc + 1) // 2   # xselfT windows < WSPLIT load early

        @block.sync
        def _(sp):
            nonlocal n_init, n_ini1b, n_ini2
            # edge/onehot stream through the rotating GO buffer; dense-path
            # weights + first xselfT half after chunk 3, the rest after
            # chunk 7. Consumer-paced issue keeps rotating-sem waits sound.
            for gi, (a, b) in enumerate(go_dmas):
                if gi >= GO_R:
                    sp.wait_ge(s_pe, go_dmas[gi - GO_R][1])
                sl = (gi % GO_R) * slot_tiles
                sp.dma_start(out=GO[:, sl * TROW: (sl + b - a) * TROW],
                             in_=go_d[:, a * TROW: b * TROW]
                             ).then_inc(s_goN[gi % GO_R], 16)
                if gi == 0:
                    def ld(dst_ap, src_ap):
                        nonlocal n_init
                        sp.dma_start(out=dst_ap, in_=src_ap).then_inc(s_init, 16)
                        n_init += 1
                    ofs = 0
                    for c, kc in enumerate(ch0):
                        ld(W0s_s[0:kc, c * n_hid:(c + 1) * n_hid], W0s_d[ofs:ofs + kc, :])
                        ld(W0n_s[0:kc, c * n_hid:(c + 1) * n_hid], W0n_d[ofs:ofs + kc, :])
                        ofs += kc
                    last = NC0 - 1
                    ld(W0s_s[ch0[last]:ch0[last] + 1, last * n_hid:(last + 1) * n_hid],
                       W0s_d[f_in:f_in + 1, :])
                    for c in range(NC0):
                        kcr = ch0[c] + (1 if c == NC0 - 1 else 0)
                        ld(xselfT[0:kcr, c * SFW: c * SFW + 3 * P],
                           xselfT_d[0:kcr, c * SFW: c * SFW + 3 * P])
                    ld(OH1[:, :], oh1_d[:, :])
                elif gi == 1:
                    def ld1b(dst_ap, src_ap):
                        nonlocal n_ini1b
                        sp.dma_start(out=dst_ap, in_=src_ap).then_inc(s_ini1b, 16)
                        n_ini1b += 1
                    for c in range(NC0):
                        kcr = ch0[c] + (1 if c == NC0 - 1 else 0)
                        ld1b(xselfT[0:kcr, c * SFW + 3 * P: c * SFW + WSPLIT * P],
                             xselfT_d[0:kcr, c * SFW + 3 * P: c * SFW + WSPLIT * P])
                elif gi == 3:
                    def ld2(dst_ap, src_ap):
                        nonlocal n_ini2
                        sp.dma_start(out=dst_ap, in_=src_ap).then_inc(s_ini2, 16)
                        n_ini2 += 1
                    for c in range(NC0):
                        kcr = ch0[c] + (1 if c == NC0 - 1 else 0)
                        ld2(xselfT[0:kcr, c * SFW + WSPLIT * P: (c + 1) * SFW],
                            xselfT_d[0:kcr, c * SFW + WSPLIT * P: (c + 1) * SFW])
                    ofs = 0
                    for c, kc in enumerate(ch1):
                        ld2(W1s_s[0:kc, c * n_cls:(c + 1) * n_cls], W1s_d[ofs:ofs + kc, :])
                        ld2(W1n_s[0:kc, c * n_cls:(c + 1) * n_cls], W1n_d[ofs:ofs + kc, :])
                        ofs += kc
                    ld2(b1row[0:1, :], W1s_d[n_hid:n_hid + 1, :])
                    ld2(ones1[0:1, :], ones1_d[0:1, :])
            sp.wait_ge(s_od, 16)

        def dense0(t_, w):
            """dense matmuls producing h window w (into ps_h[w%2])"""
            t_.wait_ge(s_cp, NC0 * (w + 1))      # copies of window w done
            if w >= 2:
                t_.wait_ge(s_hs, w - 1)          # ps_h[w%2] free (relu w-2 done)
            bb = w % 2
            k = 0
            for c in range(NC0):
                kc = ch0[c] + (1 if c == NC0 - 1 else 0)
                t_.matmul(out=ps_h[bb][0:P, 0:n_hid],
                          lhsT=xselfT[0:kc, c * SFW + w * P: c * SFW + (w + 1) * P],
                          rhs=W0s_s[0:kc, c * n_hid:(c + 1) * n_hid],
                          start=(k == 0), stop=False)
                k += 1
            for c in range(NC0):
                kc = ch0[c]
                mm = t_.matmul(out=ps_h[bb][0:P, 0:n_hid],
                               lhsT=aggT[0:kc, bb * FPAD0 + c * P: bb * FPAD0 + (c + 1) * P],
                               rhs=W0n_s[0:kc, c * n_hid:(c + 1) * n_hid],
                               start=False, stop=(k == 2 * NC0 - 1))
                k += 1
            mm.then_inc(s_wmm, 1)

        def l1tile(t_, j):
            """L1 one-hot matmul tile j: j=0 self (identity), else window j-1"""
            t_.wait_ge(s_hs, max(1, j))          # its window's relu done
            base = NC1 * P if j == 0 else 0
            hofs = 0 if j == 0 else (j - 1) * n_hid
            for c in range(NC1):
                mm = t_.matmul(
                    out=ps_l1[0:P, base + c * P: base + (c + 1) * P],
                    lhsT=h_sb[0:P, hofs + c * P: hofs + (c + 1) * P],
                    rhs=OH1[:, j * P:(j + 1) * P],
                    start=(j == 0 and c == 0),
                    stop=(j == nwc and c == NC1 - 1))
            mm.then_inc(s_l1, 1)

        @block.tensor
        def _(t_):
            gate = 0
            for w in range(nwc):
                bb = w % 2
                if w >= 2:
                    t_.wait_ge(s_cp, NC0 * (w - 1))   # ps_agg[bb] free
                for j in range(tiles_w0[w]):
                    t = cum_tiles[w] + j
                    if gate_of_tile[t] > gate:
                        gate = gate_of_tile[t]
                        gc_ = gate - 1
                        t_.wait_ge(s_goN[gc_ % GO_R], 16 * (gc_ // GO_R + 1))
                    gi, toff = chunk_of_tile[t]
                    base = ((gi % GO_R) * slot_tiles + toff) * TROW
                    first = (j == 0)
                    lastt = (j == tiles_w0[w] - 1)
                    fofs = 0
                    for c in range(NC0):
                        mc = ch0[c]
                        mm = t_.matmul(
                            out=ps_agg[bb][0:mc, c * P:(c + 1) * P],
                            lhsT=GO[:, base + fofs: base + fofs + mc],
                            rhs=GO[:, base + F_IN: base + TROW],
                            start=first and (c == first_c0[banks0[c]]),
                            stop=lastt and (c == last_c0[banks0[c]]))
                        fofs += mc
                    mm.then_inc(s_pe, 1)
                if w == 0:
                    t_.wait_ge(s_init, 16 * n_init)
                if w == 4:
                    t_.wait_ge(s_ini1b, 16 * n_ini1b)  # xselfT w3..5 in
                if w == WSPLIT + 1:
                    t_.wait_ge(s_ini2, 16 * n_ini2)   # 2nd xselfT half in
                if w >= 1:
                    dense0(t_, w - 1)
                if w >= 2:
                    l1tile(t_, w - 2)            # interleaved: relu ready
            dense0(t_, nwc - 1)
            for j in range(nwc - 2, nwc + 1):    # remaining L1 tiles
                l1tile(t_, j)
            # L1 dense
            t_.wait_ge(s_cp, NC0 * nwc + 2 * NC1)
            k = 0
            nmm = 2 * NC1 + 1
            for c in range(NC1):
                mc = ch1[c]
                t_.matmul(out=ps_out[0:dpc1, 0:n_cls],
                          lhsT=self1T[0:mc, c * P: c * P + dpc1],
                          rhs=W1s_s[0:mc, c * n_cls:(c + 1) * n_cls],
                          start=(k == 0), stop=False)
                k += 1
            t_.matmul(out=ps_out[0:dpc1, 0:n_cls],
                      lhsT=ones1[0:1, 0:dpc1],
                      rhs=b1row[0:1, 0:n_cls],
                      start=False, stop=False)
            k += 1
            for c in range(NC1):
                mc = ch1[c]
                mm = t_.matmul(out=ps_out[0:dpc1, 0:n_cls],
                               lhsT=agg1T[0:mc, c * P: c * P + dpc1],
                               rhs=W1n_s[0:mc, c * n_cls:(c + 1) * n_cls],
                               start=False, stop=(k == nmm - 1))
                k += 1
            mm.then_inc(s_wmm, 1)

        @block.scalar
        def _(s):
            for w in range(nwc):
                bb = w % 2
                s.wait_ge(s_pe, cum_tiles[w + 1])
                for c in range(NC0):
                    mc = ch0[c]
                    s.activation(out=aggT[0:mc, bb * FPAD0 + c * P: bb * FPAD0 + (c + 1) * P],
                                 in_=ps_agg[bb][0:mc, c * P:(c + 1) * P],
                                 func=AF.Copy).then_inc(s_cp, 1)
                if w >= 1:
                    s.wait_ge(s_wmm, w)
                    s.activation(out=h_sb[:, (w - 1) * n_hid: w * n_hid],
                                 in_=ps_h[(w - 1) % 2][:, :], func=AF.Relu).then_inc(s_hs, 1)
            w = nwc
            s.wait_ge(s_wmm, w)
            s.activation(out=h_sb[:, (w - 1) * n_hid: w * n_hid],
                         in_=ps_h[(w - 1) % 2][:, :], func=AF.Relu).then_inc(s_hs, 1)
            # L1 copies
            s.wait_ge(s_l1, nwc + 1)
            for c in range(NC1):
                s.activation(out=agg1T[0:P, c * P:(c + 1) * P],
                             in_=ps_l1[0:P, c * P:(c + 1) * P],
                             func=AF.Copy).then_inc(s_cp, 1)
                s.activation(out=self1T[0:P, c * P:(c + 1) * P],
                             in_=ps_l1[0:P, NC1 * P + c * P: NC1 * P + (c + 1) * P],
                             func=AF.Copy).then_inc(s_cp, 1)
            s.wait_ge(s_wmm, nwc + 1)
            s.activation(out=out_sb[0:dpc1, :], in_=ps_out[0:dpc1, :],
                         func=AF.Copy).then_inc(s_hs, 1)
            s.wait_ge(s_hs, nwc + 1)   # out_sb writes landed
            s.dma_start(out=out_d[0:dpc1, :], in_=out_sb[0:dpc1, :]).then_inc(s_od, 16)

    nc.compile()
    return nc, None


def _run(inputs, dims=None, trace=False, tmpdir=None):
    from concourse.bass_utils import run_bass_kernel_spmd
    in_maps, prm = _preprocess(**inputs)
    nc, _ = _build_nc(prm)
    res = run_bass_kernel_spmd(nc, in_maps, core_ids=list(range(NCORES)),
                               trace=trace, tmpdir=tmpdir)
    dpc1 = N_DST1 // NCORES
    out = np.concatenate([res.results[c]["out"][:dpc1] for c in range(NCORES)], 0)
    return out.astype(np.float32), res


def kernel(**inputs):
    out, _ = _run(inputs)
    return out
